# revision 16
# baseline (speedup 1.0000x reference)
"""Trainium2 Bass kernel for nn_DensePropMaxPool.

Computation (per batch b, feature h):
  map_h[b,h,s,e] = max(x[b,h,s..e]) for e>=s (upper triangle), 0 elsewhere
  map_mask[b,0,s,e] = upper-triangular ones
  props_h[b,p,h] = max(x[b,h,start_p..end_p-1])

Device strategy (data-parallel over batch, 4 batches per core):
  * One DVE tensor_tensor_scan per (b, h-chunk) tile computes the whole
    masked map tile in a stride-65 "extended" layout (pos = s*65 + d,
    d = e-s): data0 carries -1e30 resets at row starts (pos=65s) and at
    first-invalid slots (pos=64s+64); data1 is a 3-D sliding AP reading
    x[h, s+d] from a zero-padded x tile. Invalid (lower-triangle) slots
    come out exactly 0, and positions 0..4095 of the extended layout
    coincide with the DRAM [s,e] layout, so one contiguous 2MB DMA
    stores each tile.
  * Window-max tables T_k[s] = max(x[s..s+2^k-1]) are stride-65 slices
    of the scan output; PE-transposed into an [s-on-partitions] stack.
  * props: every window [s,e) = union of two 2^k windows; two one-hot
    matmul gathers (host-built one-hot operands) + one DVE max produce
    props tiles [128p, 512h] directly in the output layout. Props are
    sorted by table index on the host so most one-hot chunks are all
    zero and their matmuls are skipped; the host unpermutes at the end.
"""

import os
import numpy as np

B, H, C, P = 32, 512, 64, 1024
NCORES = 8
BL = B // NCORES          # 4 batches per core
S = C                     # 64
ST = S + 1                # 65, extended row stride
EXT = S * ST              # 4160
HC = H // 128             # 4 h-chunks per batch

_CACHE = {}


def _fix_sync_waits(nc, mybir):
    """walrus on this toolchain accepts only ONE sync-wait per instruction;
    Tile attaches several. Split extras onto injected same-engine NoOps."""
    ctr = 0
    for f in nc.m.functions:
        for blk in f.blocks:
            insts = blk.instructions
            if not any(
                i.sync_info is not None and i.sync_info.on_wait and len(i.sync_info.on_wait) > 1
                for i in insts
            ):
                continue
            new = []
            for inst in insts:
                si = inst.sync_info
                if si is not None and si.on_wait and len(si.on_wait) > 1:
                    waits = list(si.on_wait)
                    for w in waits[:-1]:
                        nop = mybir.InstNoOp(name=f"I-waitsplit-{ctr}", ins=[], outs=[])
                        ctr += 1
                        nop.engine = inst.engine
                        nop.sync_info = mybir.SyncInfo(on_wait=[w], on_update=[])
                        new.append(nop)
                    inst.sync_info = mybir.SyncInfo(
                        on_wait=[waits[-1]], on_update=list(si.on_update)
                    )
                new.append(inst)
            blk.instructions = new
    return ctr


def _build_props_plan(props):
    """Deduplicate (start, end) pairs, sort by (k, degenerate, s); build
    one-hot gather operands and the per-block matmul chunk plan. The device
    gathers only unique windows; the host scatters them back to all P
    proposals via `inv`."""
    s_all = np.asarray(props[:, 0], dtype=np.int64)
    e_all = np.asarray(props[:, 1], dtype=np.int64)
    pair_ids = s_all * (C + 1) + e_all
    uniq, inv = np.unique(pair_ids, return_inverse=True)
    s_idx = uniq // (C + 1)
    e_idx = uniq % (C + 1)
    L = e_idx - s_idx
    k = np.floor(np.log2(L)).astype(np.int64)
    b2 = e_idx - (1 << k)
    degen = (s_idx == b2)
    jj = np.where(k < 4, k, k - 4)
    order = np.lexsort((s_idx, k, jj))
    U = len(uniq)
    NB = (U + 127) // 128
    # inv_sorted[p] = row (in device output) holding prop p's value
    rank = np.empty(U, np.int64)
    rank[order] = np.arange(U)
    inv_sorted = rank[inv]
    oh = np.zeros((NB * 8, 128, 128), np.float32)
    plan = []  # per block: (used_A tuple, used_B tuple or None, degenerate)
    for blk in range(NB):
        used_A, used_B = set(), set()
        all_degen = True
        for m in range(min(128, U - blk * 128)):
            p = int(order[blk * 128 + m])
            kk = int(k[p])
            j, half = (kk, 0) if kk < 4 else (kk - 4, 1)
            oh[blk * 8 + j, half * 64 + int(s_idx[p]), m] = 1.0
            oh[blk * 8 + 4 + j, half * 64 + int(b2[p]), m] = 1.0
            used_A.add(j)
            used_B.add(j)
            if not degen[p]:
                all_degen = False
        plan.append((tuple(sorted(used_A)),
                     None if all_degen else tuple(sorted(used_B)),
                     all_degen, min(128, U - blk * 128)))
        import ml_dtypes
    ohv = np.ascontiguousarray(
        oh.transpose(1, 0, 2).reshape(128, NB * 8 * 128).astype(ml_dtypes.bfloat16))
    return ohv, inv_sorted, tuple(plan)


def _trace_program(plan):
    import concourse.bass as bass
    import concourse.mybir as mybir
    from concourse.tile import TileContext
    from concourse.masks import make_identity

    f32 = mybir.dt.float32
    nc = bass.Bass(
        "TRN2", target_bir_lowering=False, debug=False,
        enable_asserts=False, num_devices=NCORES,
    )
    NB = len(plan)
    x_l = nc.dram_tensor("x_l", [BL * H, C], f32, kind="ExternalInput")
    oh_l = nc.dram_tensor("oh_l", [128, NB * 8 * 128], mybir.dt.bfloat16, kind="ExternalInput")
    mask_in = nc.dram_tensor("mask_in", [BL, C * C], f32, kind="ExternalInput")
    map_l = nc.dram_tensor("map_l", [BL * H, C * C], f32, kind="ExternalOutput")
    props_l = nc.dram_tensor("props_l", [BL * NB * 128, H], f32, kind="ExternalOutput")
    mask_l = nc.dram_tensor("mask_l", [BL, C * C], f32, kind="ExternalOutput")

    with TileContext(nc) as tc:
        with tc.tile_pool(name="consts", bufs=1) as consts, \
             tc.tile_pool(name="maps", bufs=4) as maps, \
             tc.tile_pool(name="sb", bufs=2) as sb, \
             tc.tile_pool(name="prs", bufs=3) as prs, \
             tc.tile_pool(name="pst", bufs=3, space="PSUM") as pst, \
             tc.tile_pool(name="psg", bufs=4, space="PSUM") as psg:

            xts = []
            for i in range(3):
                xt = consts.tile([128, 128], f32, tag=f"xt{i}")
                nc.scalar.memzero(xt[:, S:128])
                xts.append(xt)

            d0 = consts.tile([128, EXT], f32, tag="d0")
            nc.scalar.memzero(d0[:])
            nc.vector.memset(d0[:, 0:EXT:ST], -1e30)
            nc.vector.memset(d0[:, S:EXT:S], -1e30)

            oht = consts.tile([128, NB * 8 * 128], f32, tag="oht")

            ident = consts.tile([128, 128], f32, tag="ident")
            make_identity(nc, ident[:])
            tsts = []
            for i in range(2):
                t = consts.tile([128, 2048], f32, tag=f"tst{i}")
                nc.gpsimd.memset(t[64:128, 1536:2048], 0.0)
                tsts.append(t)

            v = nc.vector

            def emit_tile(b, hc):
                it = b * HC + hc
                xt = xts[it % 3]
                rows = slice(b * H + hc * 128, b * H + (hc + 1) * 128)
                nc.gpsimd.dma_start(out=xt[:, 0:S], in_=x_l[rows, :])

                mp = maps.tile([128, EXT], f32, tag="mp")
                xt_ap = xt[:, 0:S]
                HALF = EXT // 2  # 2080 = 32 rows, clean row boundary
                for h2 in range(2):
                    data1 = bass.AP(
                        xt_ap.tensor, xt_ap.offset + h2 * (S // 2),
                        [list(xt_ap.ap[0]), [1, S // 2], [1, ST]],
                    )
                    v.add_instruction(
                        mybir.InstTensorScalarPtr(
                            name=nc.get_next_instruction_name(),
                            is_tensor_tensor_scan=True,
                            is_scalar_tensor_tensor=True,
                            op0=mybir.AluOpType.add,
                            op1=mybir.AluOpType.max,
                            ins=[
                                v.lower_ap(d0[:, h2 * HALF:(h2 + 1) * HALF]),
                                v.lower_ap_or_imm(0.0),
                                v.lower_ap(data1),
                            ],
                            outs=[v.lower_ap(mp[:, h2 * HALF:(h2 + 1) * HALF])],
                        )
                    )
                tst = tsts[b % 2]
                for k in range(7):
                    pt = pst.tile([64, 128], f32, tag="pt")
                    nc.tensor.transpose(
                        out=pt[:], in_=mp[:, (1 << k) - 1:EXT:ST], identity=ident[:]
                    )
                    half = 0 if k < 4 else 1
                    j = k if k < 4 else k - 4
                    nc.scalar.copy(
                        out=tst[half * 64:half * 64 + 64,
                                j * 512 + hc * 128: j * 512 + (hc + 1) * 128],
                        in_=pt[:],
                    )
                nc.sync.dma_start(out=map_l[rows, 0:EXT // 2], in_=mp[:, 0:EXT // 2])
                nc.sync.dma_start(out=map_l[rows, EXT // 2:4096], in_=mp[:, EXT // 2:4096])

            prb_acc = [consts.tile([128, 512], f32, tag=f"prb{pc}", name=f"prb{pc}") for pc in range(NB)]

            def emit_props_slice(b, pc, hc):
                # gather only h-columns [hc*128, (hc+1)*128) of batch b's
                # tables; final values go into the per-pc accumulator tile
                tst = tsts[b % 2]
                used_A, used_B, all_degen, rows_u = plan[pc]
                cols = slice(hc * 128, (hc + 1) * 128)
                pa = psg.tile([128, 512], f32, tag="pg")
                for i, j in enumerate(used_A):
                    nc.tensor.matmul(
                        out=pa[:, 0:128],
                        lhsT=oht[:, (pc * 8 + j) * 128:(pc * 8 + j + 1) * 128],
                        rhs=tst[:, j * 512 + hc * 128: j * 512 + (hc + 1) * 128],
                        start=(i == 0), stop=(i == len(used_A) - 1),
                    )
                dst = prb_acc[pc][:, cols]
                if all_degen:
                    nc.scalar.copy(out=dst, in_=pa[:, 0:128])
                else:
                    pb = psg.tile([128, 512], f32, tag="pg")
                    for i, j in enumerate(used_B):
                        nc.tensor.matmul(
                            out=pb[:, 0:128],
                            lhsT=oht[:, (pc * 8 + 4 + j) * 128:(pc * 8 + 5 + j) * 128],
                            rhs=tst[:, j * 512 + hc * 128: j * 512 + (hc + 1) * 128],
                            start=(i == 0), stop=(i == len(used_B) - 1),
                        )
                    pb_sb = sb.tile([128, 512], f32, tag="pbs")
                    nc.scalar.copy(out=pb_sb[:, 0:128], in_=pb[:, 0:128])
                    nc.vector.tensor_tensor(
                        out=dst, in0=pa[:, 0:128], in1=pb_sb[:, 0:128],
                        op=mybir.AluOpType.max,
                    )
                if hc == HC - 1:
                    nc.scalar.dma_start(
                        out=props_l[(b * NB + pc) * 128: (b * NB + pc) * 128 + rows_u, :],
                        in_=prb_acc[pc][0:rows_u, :],
                    )

            def emit_props(b, pc):
                tst = tsts[b % 2]
                used_A, used_B, all_degen, rows_u = plan[pc]
                pa = psg.tile([128, 512], f32, tag="pg")
                for i, j in enumerate(used_A):
                    nc.tensor.matmul(
                        out=pa[:],
                        lhsT=oht[:, (pc * 8 + j) * 128:(pc * 8 + j + 1) * 128],
                        rhs=tst[:, j * 512:(j + 1) * 512],
                        start=(i == 0), stop=(i == len(used_A) - 1),
                    )
                pr = prs.tile([128, 512], f32, tag="pr")
                if all_degen:
                    nc.scalar.copy(out=pr[:], in_=pa[:])
                else:
                    pb = psg.tile([128, 512], f32, tag="pg")
                    for i, j in enumerate(used_B):
                        nc.tensor.matmul(
                            out=pb[:],
                            lhsT=oht[:, (pc * 8 + 4 + j) * 128:(pc * 8 + 5 + j) * 128],
                            rhs=tst[:, j * 512:(j + 1) * 512],
                            start=(i == 0), stop=(i == len(used_B) - 1),
                        )
                    pb_sb = sb.tile([128, 512], f32, tag="pbs")
                    nc.scalar.copy(out=pb_sb[:], in_=pb[:])
                    nc.vector.tensor_tensor(
                        out=pr[:], in0=pa[:], in1=pb_sb[:], op=mybir.AluOpType.max
                    )
                nc.scalar.dma_start(
                    out=props_l[(b * NB + pc) * 128: (b * NB + pc) * 128 + rows_u, :],
                    in_=pr[0:rows_u, :],
                )

            # software-pipelined emission: batch b's scan phase interleaved
            # with batch b-1's props phase; the last batch's props run
            # per-h-slice right behind each of its own scans
            for b in range(BL):
                for hc in range(HC):
                    emit_tile(b, hc)
                    if b == 0 and hc == 0:
                        nc.gpsimd.dma_start(out=oht[:], in_=oh_l[:])
                        nc.gpsimd.dma_start(out=mask_l[:], in_=mask_in[:])
                    if b > 0:
                        for pc in range(hc * NB // HC, (hc + 1) * NB // HC):
                            emit_props(b - 1, pc)
                    if b == BL - 1 and hc > 0:
                        for pc in range(NB):
                            emit_props_slice(b, pc, hc - 1)
            for pc in range(NB):
                emit_props_slice(BL - 1, pc, HC - 1)

    _fix_sync_waits(nc, mybir)
    return nc


def kernel(x, props):
    from concourse.bass_utils import run_bass_kernel_spmd

    x = np.ascontiguousarray(np.asarray(x, dtype=np.float32))
    assert x.shape == (B, H, C)

    ohv, inv_sorted, plan = _build_props_plan(np.asarray(props))
    if _CACHE.get("plan") != plan:
        _CACHE["nc"] = _trace_program(plan)
        _CACHE["plan"] = plan
    nc = _CACHE["nc"]

    maskv = np.broadcast_to(
        np.triu(np.ones((C, C), np.float32)).reshape(1, C * C), (BL, C * C)
    ).copy()

    in_maps = []
    for i in range(NCORES):
        in_maps.append({
            "x_l": np.ascontiguousarray(x[i * BL:(i + 1) * BL].reshape(BL * H, C)),
            "oh_l": ohv,
            "mask_in": maskv,
        })

    trace = bool(int(os.environ.get("KERNEL_TRACE", "0")))
    res = run_bass_kernel_spmd(
        nc, in_maps, core_ids=list(range(NCORES)), trace=trace,
    )
    _CACHE["last_result"] = res

    map_h = np.empty((B, H, C, C), np.float32)
    props_h = np.empty((B, P, H), np.float32)
    map_mask = np.empty((B, 1, C, C), np.float32)
    for i in range(NCORES):
        r = res.results[i]
        map_h[i * BL:(i + 1) * BL] = r["map_l"].reshape(BL, H, C, C)
        nbl = r["props_l"].shape[0] // (BL * 128)
        pr = r["props_l"].reshape(BL, nbl * 128, H)
        for b in range(BL):
            props_h[i * BL + b] = pr[b][inv_sorted]
        map_mask[i * BL:(i + 1) * BL] = r["mask_l"].reshape(BL, 1, C, C)
    return props_h, map_h, map_mask


# revision 17
# speedup vs baseline: 1.0222x; 1.0222x over previous
"""Trainium2 Bass kernel for nn_DensePropMaxPool.

Computation (per batch b, feature h):
  map_h[b,h,s,e] = max(x[b,h,s..e]) for e>=s (upper triangle), 0 elsewhere
  map_mask[b,0,s,e] = upper-triangular ones
  props_h[b,p,h] = max(x[b,h,start_p..end_p-1])

Device strategy (data-parallel over batch, 4 batches per core):
  * One DVE tensor_tensor_scan per (b, h-chunk) tile computes the whole
    masked map tile in a stride-65 "extended" layout (pos = s*65 + d,
    d = e-s): data0 carries -1e30 resets at row starts (pos=65s) and at
    first-invalid slots (pos=64s+64); data1 is a 3-D sliding AP reading
    x[h, s+d] from a zero-padded x tile. Invalid (lower-triangle) slots
    come out exactly 0, and positions 0..4095 of the extended layout
    coincide with the DRAM [s,e] layout, so one contiguous 2MB DMA
    stores each tile.
  * Window-max tables T_k[s] = max(x[s..s+2^k-1]) are stride-65 slices
    of the scan output; PE-transposed into an [s-on-partitions] stack.
  * props: every window [s,e) = union of two 2^k windows; two one-hot
    matmul gathers (host-built one-hot operands) + one DVE max produce
    props tiles [128p, 512h] directly in the output layout. Props are
    sorted by table index on the host so most one-hot chunks are all
    zero and their matmuls are skipped; the host unpermutes at the end.
"""

import os
import numpy as np

B, H, C, P = 32, 512, 64, 1024
NCORES = 8
BL = B // NCORES          # 4 batches per core
S = C                     # 64
ST = S + 1                # 65, extended row stride
EXT = S * ST              # 4160
HC = H // 128             # 4 h-chunks per batch

_CACHE = {}


def _fix_sync_waits(nc, mybir):
    """walrus on this toolchain accepts only ONE sync-wait per instruction;
    Tile attaches several. Split extras onto injected same-engine NoOps."""
    ctr = 0
    for f in nc.m.functions:
        for blk in f.blocks:
            insts = blk.instructions
            if not any(
                i.sync_info is not None and i.sync_info.on_wait and len(i.sync_info.on_wait) > 1
                for i in insts
            ):
                continue
            new = []
            for inst in insts:
                si = inst.sync_info
                if si is not None and si.on_wait and len(si.on_wait) > 1:
                    waits = list(si.on_wait)
                    for w in waits[:-1]:
                        nop = mybir.InstNoOp(name=f"I-waitsplit-{ctr}", ins=[], outs=[])
                        ctr += 1
                        nop.engine = inst.engine
                        nop.sync_info = mybir.SyncInfo(on_wait=[w], on_update=[])
                        new.append(nop)
                    inst.sync_info = mybir.SyncInfo(
                        on_wait=[waits[-1]], on_update=list(si.on_update)
                    )
                new.append(inst)
            blk.instructions = new
    return ctr


def _build_props_plan(props):
    """Deduplicate (start, end) pairs, sort by (k, degenerate, s); build
    one-hot gather operands and the per-block matmul chunk plan. The device
    gathers only unique windows; the host scatters them back to all P
    proposals via `inv`."""
    s_all = np.asarray(props[:, 0], dtype=np.int64)
    e_all = np.asarray(props[:, 1], dtype=np.int64)
    pair_ids = s_all * (C + 1) + e_all
    uniq, inv = np.unique(pair_ids, return_inverse=True)
    s_idx = uniq // (C + 1)
    e_idx = uniq % (C + 1)
    L = e_idx - s_idx
    k = np.floor(np.log2(L)).astype(np.int64)
    b2 = e_idx - (1 << k)
    degen = (s_idx == b2)
    jj = np.where(k < 4, k, k - 4)
    order = np.lexsort((s_idx, k, jj))
    U = len(uniq)
    NB = (U + 127) // 128
    # inv_sorted[p] = row (in device output) holding prop p's value
    rank = np.empty(U, np.int64)
    rank[order] = np.arange(U)
    inv_sorted = rank[inv]
    oh = np.zeros((NB * 8, 128, 128), np.float32)
    plan = []  # per block: (used_A tuple, used_B tuple or None, degenerate)
    for blk in range(NB):
        used_A, used_B = set(), set()
        all_degen = True
        for m in range(min(128, U - blk * 128)):
            p = int(order[blk * 128 + m])
            kk = int(k[p])
            j, half = (kk, 0) if kk < 4 else (kk - 4, 1)
            oh[blk * 8 + j, half * 64 + int(s_idx[p]), m] = 1.0
            oh[blk * 8 + 4 + j, half * 64 + int(b2[p]), m] = 1.0
            used_A.add(j)
            used_B.add(j)
            if not degen[p]:
                all_degen = False
        plan.append((tuple(sorted(used_A)),
                     None if all_degen else tuple(sorted(used_B)),
                     all_degen, min(128, U - blk * 128)))
        import ml_dtypes
    ohv = np.ascontiguousarray(
        oh.transpose(1, 0, 2).reshape(128, NB * 8 * 128).astype(ml_dtypes.bfloat16))
    return ohv, inv_sorted, tuple(plan)


def _trace_program(plan):
    import concourse.bass as bass
    import concourse.mybir as mybir
    from concourse.tile import TileContext
    from concourse.masks import make_identity

    f32 = mybir.dt.float32
    nc = bass.Bass(
        "TRN2", target_bir_lowering=False, debug=False,
        enable_asserts=False, num_devices=NCORES,
    )
    NB = len(plan)
    x_l = nc.dram_tensor("x_l", [BL * H, C], f32, kind="ExternalInput")
    oh_l = nc.dram_tensor("oh_l", [128, NB * 8 * 128], mybir.dt.bfloat16, kind="ExternalInput")
    mask_in = nc.dram_tensor("mask_in", [BL, C * C], f32, kind="ExternalInput")
    map_l = nc.dram_tensor("map_l", [BL * H, C * C], f32, kind="ExternalOutput")
    props_l = nc.dram_tensor("props_l", [BL * NB * 128, H], f32, kind="ExternalOutput")
    mask_l = nc.dram_tensor("mask_l", [BL, C * C], f32, kind="ExternalOutput")

    with TileContext(nc) as tc:
        with tc.tile_pool(name="consts", bufs=1) as consts, \
             tc.tile_pool(name="maps", bufs=5) as maps, \
             tc.tile_pool(name="sb", bufs=2) as sb, \
             tc.tile_pool(name="prs", bufs=3) as prs, \
             tc.tile_pool(name="pst", bufs=3, space="PSUM") as pst, \
             tc.tile_pool(name="psg", bufs=4, space="PSUM") as psg:

            xts = []
            for i in range(3):
                xt = consts.tile([128, 128], f32, tag=f"xt{i}")
                nc.scalar.memzero(xt[:, S:128])
                xts.append(xt)

            d0 = consts.tile([128, EXT], f32, tag="d0")
            nc.scalar.memzero(d0[:])
            nc.vector.memset(d0[:, 0:EXT:ST], -1e30)
            nc.vector.memset(d0[:, S:EXT:S], -1e30)

            oht = consts.tile([128, NB * 8 * 128], f32, tag="oht")

            ident = consts.tile([128, 128], f32, tag="ident")
            make_identity(nc, ident[:])
            tsts = []
            for i in range(2):
                t = consts.tile([128, 2048], f32, tag=f"tst{i}")
                nc.gpsimd.memset(t[64:128, 1536:2048], 0.0)
                tsts.append(t)

            v = nc.vector

            def emit_tile(b, hc):
                it = b * HC + hc
                xt = xts[it % 3]
                rows = slice(b * H + hc * 128, b * H + (hc + 1) * 128)
                nc.gpsimd.dma_start(out=xt[:, 0:S], in_=x_l[rows, :])

                mp = maps.tile([128, EXT], f32, tag="mp")
                xt_ap = xt[:, 0:S]
                # four independent scan segments (each starts at a row
                # boundary, where the data0 reset makes initial=0 exact)
                HALF = EXT // 4  # 1040 = 16 rows
                for h2 in range(4):
                    data1 = bass.AP(
                        xt_ap.tensor, xt_ap.offset + h2 * (S // 4),
                        [list(xt_ap.ap[0]), [1, S // 4], [1, ST]],
                    )
                    v.add_instruction(
                        mybir.InstTensorScalarPtr(
                            name=nc.get_next_instruction_name(),
                            is_tensor_tensor_scan=True,
                            is_scalar_tensor_tensor=True,
                            op0=mybir.AluOpType.add,
                            op1=mybir.AluOpType.max,
                            ins=[
                                v.lower_ap(d0[:, h2 * HALF:(h2 + 1) * HALF]),
                                v.lower_ap_or_imm(0.0),
                                v.lower_ap(data1),
                            ],
                            outs=[v.lower_ap(mp[:, h2 * HALF:(h2 + 1) * HALF])],
                        )
                    )
                tst = tsts[b % 2]
                for k in range(7):
                    pt = pst.tile([64, 128], f32, tag="pt")
                    nc.tensor.transpose(
                        out=pt[:], in_=mp[:, (1 << k) - 1:EXT:ST], identity=ident[:]
                    )
                    half = 0 if k < 4 else 1
                    j = k if k < 4 else k - 4
                    nc.scalar.copy(
                        out=tst[half * 64:half * 64 + 64,
                                j * 512 + hc * 128: j * 512 + (hc + 1) * 128],
                        in_=pt[:],
                    )
                for h2 in range(4):
                    lo = h2 * (EXT // 4)
                    hi = min((h2 + 1) * (EXT // 4), 4096)
                    nc.sync.dma_start(out=map_l[rows, lo:hi], in_=mp[:, lo:hi])

            prb_acc = [consts.tile([128, 512], f32, tag=f"prb{pc}", name=f"prb{pc}") for pc in range(NB)]

            def emit_props_slice(b, pc, hc):
                # gather only h-columns [hc*128, (hc+1)*128) of batch b's
                # tables; final values go into the per-pc accumulator tile
                tst = tsts[b % 2]
                used_A, used_B, all_degen, rows_u = plan[pc]
                cols = slice(hc * 128, (hc + 1) * 128)
                pa = psg.tile([128, 512], f32, tag="pg")
                for i, j in enumerate(used_A):
                    nc.tensor.matmul(
                        out=pa[:, 0:128],
                        lhsT=oht[:, (pc * 8 + j) * 128:(pc * 8 + j + 1) * 128],
                        rhs=tst[:, j * 512 + hc * 128: j * 512 + (hc + 1) * 128],
                        start=(i == 0), stop=(i == len(used_A) - 1),
                    )
                dst = prb_acc[pc][:, cols]
                if all_degen:
                    nc.scalar.copy(out=dst, in_=pa[:, 0:128])
                else:
                    pb = psg.tile([128, 512], f32, tag="pg")
                    for i, j in enumerate(used_B):
                        nc.tensor.matmul(
                            out=pb[:, 0:128],
                            lhsT=oht[:, (pc * 8 + 4 + j) * 128:(pc * 8 + 5 + j) * 128],
                            rhs=tst[:, j * 512 + hc * 128: j * 512 + (hc + 1) * 128],
                            start=(i == 0), stop=(i == len(used_B) - 1),
                        )
                    pb_sb = sb.tile([128, 512], f32, tag="pbs")
                    nc.scalar.copy(out=pb_sb[:, 0:128], in_=pb[:, 0:128])
                    nc.vector.tensor_tensor(
                        out=dst, in0=pa[:, 0:128], in1=pb_sb[:, 0:128],
                        op=mybir.AluOpType.max,
                    )
                if hc == HC - 1:
                    nc.scalar.dma_start(
                        out=props_l[(b * NB + pc) * 128: (b * NB + pc) * 128 + rows_u, :],
                        in_=prb_acc[pc][0:rows_u, :],
                    )

            def emit_props(b, pc):
                tst = tsts[b % 2]
                used_A, used_B, all_degen, rows_u = plan[pc]
                pa = psg.tile([128, 512], f32, tag="pg")
                for i, j in enumerate(used_A):
                    nc.tensor.matmul(
                        out=pa[:],
                        lhsT=oht[:, (pc * 8 + j) * 128:(pc * 8 + j + 1) * 128],
                        rhs=tst[:, j * 512:(j + 1) * 512],
                        start=(i == 0), stop=(i == len(used_A) - 1),
                    )
                pr = prs.tile([128, 512], f32, tag="pr")
                if all_degen:
                    nc.scalar.copy(out=pr[:], in_=pa[:])
                else:
                    pb = psg.tile([128, 512], f32, tag="pg")
                    for i, j in enumerate(used_B):
                        nc.tensor.matmul(
                            out=pb[:],
                            lhsT=oht[:, (pc * 8 + 4 + j) * 128:(pc * 8 + 5 + j) * 128],
                            rhs=tst[:, j * 512:(j + 1) * 512],
                            start=(i == 0), stop=(i == len(used_B) - 1),
                        )
                    pb_sb = sb.tile([128, 512], f32, tag="pbs")
                    nc.scalar.copy(out=pb_sb[:], in_=pb[:])
                    nc.vector.tensor_tensor(
                        out=pr[:], in0=pa[:], in1=pb_sb[:], op=mybir.AluOpType.max
                    )
                nc.scalar.dma_start(
                    out=props_l[(b * NB + pc) * 128: (b * NB + pc) * 128 + rows_u, :],
                    in_=pr[0:rows_u, :],
                )

            # software-pipelined emission: batch b's scan phase interleaved
            # with batch b-1's props phase; the last batch's props run
            # per-h-slice right behind each of its own scans
            for b in range(BL):
                for hc in range(HC):
                    emit_tile(b, hc)
                    if b == 0 and hc == 0:
                        nc.gpsimd.dma_start(out=oht[:], in_=oh_l[:])
                        nc.gpsimd.dma_start(out=mask_l[:], in_=mask_in[:])
                    if b > 0:
                        for pc in range(hc * NB // HC, (hc + 1) * NB // HC):
                            emit_props(b - 1, pc)
                    if b == BL - 1 and hc > 0:
                        for pc in range(NB):
                            emit_props_slice(b, pc, hc - 1)
            for pc in range(NB):
                emit_props_slice(BL - 1, pc, HC - 1)

    _fix_sync_waits(nc, mybir)
    return nc


def kernel(x, props):
    from concourse.bass_utils import run_bass_kernel_spmd

    x = np.ascontiguousarray(np.asarray(x, dtype=np.float32))
    assert x.shape == (B, H, C)

    ohv, inv_sorted, plan = _build_props_plan(np.asarray(props))
    if _CACHE.get("plan") != plan:
        _CACHE["nc"] = _trace_program(plan)
        _CACHE["plan"] = plan
    nc = _CACHE["nc"]

    maskv = np.broadcast_to(
        np.triu(np.ones((C, C), np.float32)).reshape(1, C * C), (BL, C * C)
    ).copy()

    in_maps = []
    for i in range(NCORES):
        in_maps.append({
            "x_l": np.ascontiguousarray(x[i * BL:(i + 1) * BL].reshape(BL * H, C)),
            "oh_l": ohv,
            "mask_in": maskv,
        })

    trace = bool(int(os.environ.get("KERNEL_TRACE", "0")))
    res = run_bass_kernel_spmd(
        nc, in_maps, core_ids=list(range(NCORES)), trace=trace,
    )
    _CACHE["last_result"] = res

    map_h = np.empty((B, H, C, C), np.float32)
    props_h = np.empty((B, P, H), np.float32)
    map_mask = np.empty((B, 1, C, C), np.float32)
    for i in range(NCORES):
        r = res.results[i]
        map_h[i * BL:(i + 1) * BL] = r["map_l"].reshape(BL, H, C, C)
        nbl = r["props_l"].shape[0] // (BL * 128)
        pr = r["props_l"].reshape(BL, nbl * 128, H)
        for b in range(BL):
            props_h[i * BL + b] = pr[b][inv_sorted]
        map_mask[i * BL:(i + 1) * BL] = r["mask_l"].reshape(BL, 1, C, C)
    return props_h, map_h, map_mask
